# revision 1
# baseline (speedup 1.0000x reference)
"""Cached causal self-attention (single-token decode) on 8 Trainium2 cores.

Sharding: tensor-parallel over heads. Each core owns 4 of the 32 heads:
 - W_qkv rows / b_qkv entries for its heads (q,k,v stacked -> 1536 rows)
 - the KV cache slice for its heads
 - W_out columns for its heads' dims
Each core computes a partial y (16,4096); host sums partials and adds b_out.

On-core dataflow (all fp32):
 - qkv = x @ Wslice.T + bslice on PE (weights pre-transposed on host)
 - per (batch,head) pair: q broadcast via selector-matrix matmul (PSUM),
   scores computed on DVE as tensor_tensor_reduce(K_tile * q_bcast) ->
   column layout (128 seq positions per column); K streamed in natural
   layout, so no transposes anywhere on the critical path.
 - new token's k/v are DMA'd into row 127 of the last (127-row) cache tile,
   making all 32 tiles uniform; softmax needs no max-subtraction (scores
   bounded), exp+per-partition-sum fused in one ScalarE activation.
 - A@V on PE with V natural as stationary, exp(scores) column as moving ->
   output lands as (128 hd, 64 pairs) which is exactly the lhsT layout the
   output projection needs. Denominators via ones-matmul column sum.
"""

import math
from contextlib import ExitStack

import numpy as np

B = 16
H = 32
HD = 128
D = 4096
S_PRIOR = 4095
N_CORES = 8
HC = H // N_CORES          # heads per core
EQ = HC * HD               # 512: per-core q (or k or v) width
E3 = 3 * EQ                # 1536
SCALE = float(1.0 / np.float32(np.sqrt(np.float32(HD))))
F32 = None  # set after mybir import


def build(b=B, hc=HC, d=D, s_prior=S_PRIOR, reps=1, debug_stage=4):
    import concourse.bass as bass
    import concourse.mybir as mybir
    import concourse.tile as tile
    from concourse import bacc
    from concourse.masks import make_identity

    def bcast_mid(ap2d, n):
        """(P, F) AP -> (P, n, F) AP with a step-0 middle dim (free broadcast)."""
        return bass.AP(
            tensor=ap2d.tensor, offset=ap2d.offset,
            ap=[ap2d.ap[0], [0, n], ap2d.ap[1]],
        )

    f32 = mybir.dt.float32
    eq = hc * HD
    e3 = 3 * eq
    npairs = hc * b
    assert (s_prior + 1) % 128 == 0
    n_tiles = (s_prior + 1) // 128      # 32
    n_full = n_tiles - 1                # full 128-row cache tiles
    tail_rows = 127                     # cache rows in last tile (+1 new)
    nd = d // 128                       # d-tiles for projections
    CH = 512                            # psum free chunk
    nch = (e3 + CH - 1) // CH           # qkv out chunks
    nyj = (d + CH - 1) // CH            # out-proj chunks
    # group full cache tiles into supertiles of <=16 tiles (1 MB DMAs)
    groups = []
    t0 = 0
    while t0 < n_full:
        cnt = min(16, n_full - t0)
        groups.append((t0, cnt))
        t0 += cnt

    nc = bacc.Bacc(trn_type="TRN2")
    xT = nc.dram_tensor("xT", [d, b], f32, kind="ExternalInput")
    wqkvT = nc.dram_tensor("wqkvT", [d, e3], f32, kind="ExternalInput")
    bqkv = nc.dram_tensor("bqkv", [1, e3], f32, kind="ExternalInput")
    kc = nc.dram_tensor("kc", [b, hc, s_prior, HD], f32, kind="ExternalInput")
    vc = nc.dram_tensor("vc", [b, hc, s_prior, HD], f32, kind="ExternalInput")
    woutT = nc.dram_tensor("woutT", [eq, d], f32, kind="ExternalInput")
    y = nc.dram_tensor("y", [b, d], f32, kind="ExternalOutput")

    mult = mybir.AluOpType.mult
    addop = mybir.AluOpType.add

    with tile.TileContext(nc) as tc, ExitStack() as ctx:
        consts = ctx.enter_context(tc.tile_pool(name="consts", bufs=1))
        xsp = ctx.enter_context(tc.tile_pool(name="xsp", bufs=1))
        qkvp = ctx.enter_context(tc.tile_pool(name="qkvp", bufs=1))
        ksp = ctx.enter_context(tc.tile_pool(name="ksp", bufs=3))
        vsp = ctx.enter_context(tc.tile_pool(name="vsp", bufs=3))
        ktp = ctx.enter_context(tc.tile_pool(name="ktp", bufs=3))
        vtp = ctx.enter_context(tc.tile_pool(name="vtp", bufs=3))
        scp = ctx.enter_context(tc.tile_pool(name="scp", bufs=6))
        wp = ctx.enter_context(tc.tile_pool(name="wp", bufs=4))
        junkp = ctx.enter_context(tc.tile_pool(name="junkp", bufs=1))
        statp = ctx.enter_context(tc.tile_pool(name="statp", bufs=1))
        woutp = ctx.enter_context(tc.tile_pool(name="woutp", bufs=hc))
        ychp = ctx.enter_context(tc.tile_pool(name="ychp", bufs=2))
        miscp = ctx.enter_context(tc.tile_pool(name="miscp", bufs=1))
        pavt = ctx.enter_context(tc.tile_pool(name="pavt", bufs=1, space="PSUM"))
        pqb = ctx.enter_context(tc.tile_pool(name="pqb", bufs=1, space="PSUM"))
        pmisc = ctx.enter_context(tc.tile_pool(name="pmisc", bufs=1, space="PSUM"))
        pyp = ctx.enter_context(tc.tile_pool(name="pyp", bufs=2, space="PSUM"))

        def body():
            # ---- constants ----
            ident = consts.tile([128, 128], f32, tag="ident")
            make_identity(nc, ident[:])
            ones_row = consts.tile([1, 128], f32, tag="ones_row")
            nc.vector.memset(ones_row[:], 1.0)
            ones_col = consts.tile([128, 1], f32, tag="ones_col")
            nc.vector.memset(ones_col[:], 1.0)
            # sel_all[k, bb*128 + m] = 1 iff k == bb  (b x b*128)
            sel_all = consts.tile([b, b, 128], f32, tag="sel_all")
            nc.gpsimd.memset(sel_all[:], 0.0)
            nc.gpsimd.affine_select(
                out=sel_all[:],
                in_=sel_all[:],
                compare_op=mybir.AluOpType.not_equal,
                fill=1.0,
                base=0,
                pattern=[[1, b], [0, 128]],
                channel_multiplier=-1,
            )

            # ---- phase 1: qkv = x @ Wslice.T + b ----
            xs = xsp.tile([128, nd, b], f32, tag="xs")
            nc.sync.dma_start(out=xs[:], in_=xT.rearrange("(i p) b -> p i b", p=128))
            bq_sb = consts.tile([1, e3], f32, tag="bq")
            nc.sync.dma_start(out=bq_sb[:], in_=bqkv[:])
            qkv_sb = qkvp.tile([b, e3], f32, tag="qkv")
            with tc.tile_pool(name="wqp", bufs=3) as wqp, tc.tile_pool(
                name="pqkv", bufs=nch, space="PSUM"
            ) as pqkv:
                psq = [
                    pqkv.tile([b, min(CH, e3 - j * CH)], f32, name="psq", tag="psq")
                    for j in range(nch)
                ]
                for i in range(nd):
                    strip = wqp.tile([128, e3], f32, tag="wq_strip")
                    nc.sync.dma_start(
                        out=strip[:], in_=wqkvT[128 * i : 128 * (i + 1), :]
                    )
                    for j in range(nch):
                        w = min(CH, e3 - j * CH)
                        nc.tensor.matmul(
                            psq[j][:],
                            lhsT=xs[:, i, :],
                            rhs=strip[:, j * CH : j * CH + w],
                            start=(i == 0),
                            stop=False,
                        )
                for j in range(nch):
                    w = min(CH, e3 - j * CH)
                    nc.tensor.matmul(
                        psq[j][:],
                        lhsT=ones_row[:, :b],
                        rhs=bq_sb[:, j * CH : j * CH + w],
                        start=False,
                        stop=True,
                    )
                for j in range(nch):
                    w = min(CH, e3 - j * CH)
                    nc.vector.tensor_copy(qkv_sb[:, j * CH : j * CH + w], psq[j][:])

            if debug_stage <= 1:
                ych0 = ychp.tile([b, CH], f32, tag="ych")
                nc.vector.tensor_copy(ych0[:, :CH], qkv_sb[:, :CH])
                nc.sync.dma_start(out=y[:, :CH], in_=ych0[:, :CH])
                return

            # ---- phase 2: attention over (head, batch) pairs ----
            psum_avT = pavt.tile([128, npairs], f32, tag="avt")
            stats = statp.tile([128, npairs], f32, tag="stats")
            nc.vector.memset(stats[:], 0.0)

            for p in range(npairs if debug_stage >= 3 else 1):
                hh, bb = divmod(p, b)
                qb = pqb.tile([128, 128], f32, tag="qb")
                nc.tensor.matmul(
                    qb[:],
                    lhsT=sel_all[:, bb, :],
                    rhs=qkv_sb[:, hh * HD : (hh + 1) * HD],
                    start=True,
                    stop=True,
                )

                ksups = []
                for (g0, cnt) in groups:
                    kt = ksp.tile([128, 16, HD], f32, tag="ksup")
                    nc.sync.dma_start(
                        out=kt[:, :cnt, :],
                        in_=kc[bb, hh][128 * g0 : 128 * (g0 + cnt), :].rearrange(
                            "(i p) e -> p i e", p=128
                        ),
                    )
                    ksups.append(kt)
                ktail = ktp.tile([128, HD], f32, tag="ktail")
                nc.sync.dma_start(
                    out=ktail[:tail_rows, :], in_=kc[bb, hh][n_full * 128 :, :]
                )
                nc.sync.dma_start(
                    out=ktail[127:128, :],
                    in_=qkv_sb[bb : bb + 1, eq + hh * HD : eq + (hh + 1) * HD],
                )

                scores = scp.tile([128, n_tiles], f32, tag="scores")
                for gi, (g0, cnt) in enumerate(groups):
                    kt = ksups[gi]
                    junk = junkp.tile([128, 16, 128], f32, name="junk", tag="junk")
                    nc.vector.tensor_mul(
                        junk[:, :cnt, :], kt[:, :cnt, :], bcast_mid(qb[:], cnt)
                    )
                    nc.vector.tensor_reduce(
                        out=scores[:, g0 : g0 + cnt],
                        in_=junk[:, :cnt, :],
                        axis=mybir.AxisListType.X,
                        op=addop,
                    )
                junk2 = junkp.tile([128, 16, 128], f32, name="junk2", tag="junk")
                nc.vector.tensor_mul(junk2[:, 0, :], ktail[:], qb[:])
                nc.vector.tensor_reduce(
                    out=scores[:, n_full : n_full + 1],
                    in_=junk2[:, 0:1, :],
                    axis=mybir.AxisListType.X,
                    op=addop,
                )

                wt = wp.tile([128, n_tiles], f32, tag="wt")
                nc.scalar.activation(
                    wt[:],
                    scores[:],
                    mybir.ActivationFunctionType.Exp,
                    scale=SCALE,
                    accum_out=stats[:, p : p + 1],
                )

                vsups = []
                for (g0, cnt) in groups:
                    vt = vsp.tile([128, 16, HD], f32, tag="vsup")
                    nc.sync.dma_start(
                        out=vt[:, :cnt, :],
                        in_=vc[bb, hh][128 * g0 : 128 * (g0 + cnt), :].rearrange(
                            "(i p) e -> p i e", p=128
                        ),
                    )
                    vsups.append(vt)
                vtail = vtp.tile([128, HD], f32, tag="vtail")
                nc.sync.dma_start(
                    out=vtail[:tail_rows, :], in_=vc[bb, hh][n_full * 128 :, :]
                )
                nc.sync.dma_start(
                    out=vtail[127:128, :],
                    in_=qkv_sb[bb : bb + 1, 2 * eq + hh * HD : 2 * eq + (hh + 1) * HD],
                )

                for (g0, cnt) in groups:
                    vt = vsups[groups.index((g0, cnt))]
                    for j in range(cnt):
                        t = g0 + j
                        nc.tensor.matmul(
                            psum_avT[:, p : p + 1],
                            lhsT=vt[:, j, :],
                            rhs=wt[:, t : t + 1],
                            start=(t == 0),
                            stop=False,
                        )
                nc.tensor.matmul(
                    psum_avT[:, p : p + 1],
                    lhsT=vtail[:],
                    rhs=wt[:, n_full : n_full + 1],
                    start=False,
                    stop=True,
                )

            if debug_stage <= 3:
                ych1 = ychp.tile([b, CH], f32, tag="ych")
                nc.vector.tensor_copy(ych1[:, :npairs], stats[:b, :])
                nc.sync.dma_start(out=y[:, :npairs], in_=ych1[:, :npairs])
                return

            # ---- phase 3: denominators + output projection ----
            denom_ps = pmisc.tile([npairs, 1], f32, tag="pm")
            nc.tensor.matmul(
                denom_ps[:], lhsT=stats[:], rhs=ones_col[:], start=True, stop=True
            )
            denom_sb = miscp.tile([npairs, 1], f32, tag="denom")
            nc.vector.tensor_copy(denom_sb[:], denom_ps[:])
            recip_sb = miscp.tile([npairs, 1], f32, tag="recip")
            nc.vector.reciprocal(recip_sb[:], denom_sb[:])
            recipT_ps = pmisc.tile([1, npairs], f32, tag="pm")
            nc.tensor.transpose(
                recipT_ps[:], recip_sb[:], ident[:npairs, :npairs]
            )
            recipT_sb = miscp.tile([1, npairs], f32, tag="recipT")
            nc.vector.tensor_copy(recipT_sb[:], recipT_ps[:])
            rbc_ps = pmisc.tile([128, npairs], f32, tag="pm")
            nc.tensor.matmul(
                rbc_ps[:], lhsT=ones_row[:], rhs=recipT_sb[:], start=True, stop=True
            )
            rbc_sb = miscp.tile([128, npairs], f32, tag="rbc")
            nc.vector.tensor_copy(rbc_sb[:], rbc_ps[:])
            outT_sb = miscp.tile([128, npairs], f32, tag="outT")
            nc.vector.tensor_mul(outT_sb[:], psum_avT[:], rbc_sb[:])

            wstrips = []
            for i in range(hc):
                ws = woutp.tile([128, d], f32, tag="wout_strip")
                nc.sync.dma_start(out=ws[:], in_=woutT[128 * i : 128 * (i + 1), :])
                wstrips.append(ws)
            for j in range(nyj):
                w = min(CH, d - j * CH)
                psy = pyp.tile([b, CH], f32, tag="py")
                for i in range(hc):
                    nc.tensor.matmul(
                        psy[:, :w],
                        lhsT=outT_sb[:, i * b : (i + 1) * b],
                        rhs=wstrips[i][:, j * CH : j * CH + w],
                        start=(i == 0),
                        stop=(i == hc - 1),
                    )
                ych = ychp.tile([b, CH], f32, tag="ych")
                nc.vector.tensor_copy(ych[:, :w], psy[:, :w])
                nc.sync.dma_start(out=y[:, j * CH : j * CH + w], in_=ych[:, :w])

        if reps == 1:
            body()
        else:
            with tc.For_i(0, reps, 1):
                body()

    nc.compile()
    return nc


def shard_inputs(x_t, k_cache, v_cache, W_qkv, b_qkv, W_out, b_out):
    """Build the 8 per-core input dicts (host-side layout prep)."""
    xTc = np.ascontiguousarray(x_t.reshape(B, D).T)  # (D, B)
    in_maps = []
    for c in range(N_CORES):
        hs = slice(HC * c, HC * (c + 1))
        rq = slice(EQ * c, EQ * (c + 1))
        rk = slice(D + EQ * c, D + EQ * (c + 1))
        rv = slice(2 * D + EQ * c, 2 * D + EQ * (c + 1))
        w_slice = np.concatenate([W_qkv[rq], W_qkv[rk], W_qkv[rv]], axis=0)  # (E3, D)
        b_slice = np.concatenate([b_qkv[rq], b_qkv[rk], b_qkv[rv]])  # (E3,)
        in_maps.append(
            {
                "xT": xTc,
                "wqkvT": np.ascontiguousarray(w_slice.T),  # (D, E3)
                "bqkv": np.ascontiguousarray(b_slice.reshape(1, E3)),
                "kc": np.ascontiguousarray(k_cache[:, hs]),  # (B,HC,S_PRIOR,HD)
                "vc": np.ascontiguousarray(v_cache[:, hs]),
                "woutT": np.ascontiguousarray(W_out[:, EQ * c : EQ * (c + 1)].T),
            }
        )
    return in_maps


_CACHED_NC = None


def kernel(x_t, k_cache, v_cache, W_qkv, b_qkv, W_out, b_out):
    from concourse.bass_utils import run_bass_kernel_spmd

    global _CACHED_NC
    if _CACHED_NC is None:
        _CACHED_NC = build()
    nc = _CACHED_NC

    x_t = np.asarray(x_t, dtype=np.float32)
    k_cache = np.asarray(k_cache, dtype=np.float32)
    v_cache = np.asarray(v_cache, dtype=np.float32)
    W_qkv = np.asarray(W_qkv, dtype=np.float32)
    b_qkv = np.asarray(b_qkv, dtype=np.float32)
    W_out = np.asarray(W_out, dtype=np.float32)
    b_out = np.asarray(b_out, dtype=np.float32)

    in_maps = shard_inputs(x_t, k_cache, v_cache, W_qkv, b_qkv, W_out, b_out)
    res = run_bass_kernel_spmd(nc, in_maps, core_ids=list(range(N_CORES)))
    y = np.zeros((B, D), np.float64)
    for r in res.results:
        y += r["y"].astype(np.float64)
    y = (y + b_out.astype(np.float64)).astype(np.float32)
    return y.reshape(B, 1, D)

